# revision 31
# baseline (speedup 1.0000x reference)
"""DeBERTa-bare Trainium2 Bass kernel, v2: sequence-parallel TP.

Topology: 8 NeuronCores = 4 data-parallel pairs (one batch element each) x
2-way sequence parallel.  Each core owns HALF the tokens (rank r of the
pair owns global tokens [r*512, r*512+512)) and computes ALL 16 heads and
the FULL FFN for its token half.  Attention needs K/V over the full
sequence, so the only cross-core exchange is ONE bf16 AllGather of the
post-LN2 hidden state per layer (2 MB payload) -- no AllReduce, no
partial-sum f32 dance, and LayerNorm is fully local (no DRAM bounce).

The program is identical on every core (SPMD); all rank dependence lives
in host-prepared inputs: own-half ids/segment/mask, and the clamp-extended
relative-position projection tables with the rank's +-512 token offset
baked in:
    poskt[v] = pos_k[clip(1023 + 512 r - v)]     (c2p, q-side rows)
    posqt[w] = pos_q[clip(w - 511 + 512 r)]      (p2c, k-side rows)

DeBERTa's disentangled-attention gathers are realized as affine "skew"
access-pattern DMA reads from DRAM-resident fp8 (x256) tables, injected
into the score PSUM via scaled-identity matmuls (same mechanism as v1,
re-derived for the q-half/k-full tile geometry: cq is [512, 1152] per
head, ck is [1024, 640] per head).
"""

import sys

for _p in ("/opt/trn_rl_repo",):
    if _p not in sys.path:
        sys.path.insert(0, _p)

import numpy as np
import ml_dtypes

import concourse.bass as bass
import concourse.bacc as bacc
import concourse.tile as tile
import concourse.mybir as mybir
from concourse.masks import make_identity

F32 = mybir.dt.float32
BF16 = mybir.dt.bfloat16
FP8 = mybir.dt.float8e4
I16 = mybir.dt.int16

AF = mybir.ActivationFunctionType
OP = mybir.AluOpType

NEG = -1e9


def mm_acc(nc, ps, lhsT3, rhs3, nsub, start, stop):
    """Accumulating matmul over `nsub` 128-contraction subtiles.
    lhsT3/rhs3: APs shaped [128, nsub, *]."""
    for s in range(nsub):
        nc.tensor.matmul(ps, lhsT3[:, s], rhs3[:, s],
                         start=(start and s == 0), stop=(stop and s == nsub - 1))


def mm_accl(nc, ps, lhsT3, rhs_list, nsub, start, stop):
    """mm_acc with the rhs given as a list of per-subtile [128, N] APs."""
    for s in range(nsub):
        nc.tensor.matmul(ps, lhsT3[:, s], rhs_list[s],
                         start=(start and s == 0), stop=(stop and s == nsub - 1))


class Cfg:
    def __init__(self, B=4, S=1024, D=1024, H=16, F=4096, L=4, V=32000, SPAN=512,
                 n_cores=8, act="gelu", no_cc=False):
        self.B, self.S, self.D, self.H, self.F, self.L, self.V, self.SPAN = (
            B, S, D, H, F, L, V, SPAN)
        self.n_cores = n_cores
        self.DH = D // H
        assert self.DH == 64
        self.S2 = S // 2            # own token half
        self.DT = D // 128          # d tiles
        self.TTq = self.S2 // 128   # own token tiles
        self.TTk = S // 128         # global token tiles
        self.JT = D // 128          # head-dim col tiles (2 heads per tile)
        self.FT = F // 128          # ffn col tiles (full F)
        self.W2Q = S + 128          # cq skew table width
        self.W2K = self.S2 + 128    # ck skew table width
        self.PW = 1536              # host pos tables, clamp-extended
        self.scale = 1.0 / np.sqrt(3.0 * self.DH)
        self.act = act
        self.no_cc = no_cc
        # kept for test.py compat (unused in v2)
        self.ar_chunked = False
        self.ar_f32 = True


def build_nc(cfg):
    c = cfg
    S, S2, D = c.S, c.S2, c.D
    DT, TTq, TTk, JT, FT = c.DT, c.TTq, c.TTk, c.JT, c.FT
    nc = bacc.Bacc("TRN2", target_bir_lowering=False, debug=False,
                   num_devices=c.n_cores)

    def inp(name, shape, dt):
        return nc.dram_tensor(name, list(shape), dt, kind="ExternalInput")

    ids16 = inp("ids16", [128, S2 // 16], I16)
    tok_emb = inp("tok_emb", [c.V, D], F32)
    segsel = inp("segsel", [128, TTq], F32)
    seg0rep = inp("seg0rep", [128, D], F32)
    segdrep = inp("segdrep", [128, D], F32)
    maskt = inp("maskt", [128, TTq], F32)
    maskbias = inp("maskbias", [128, TTk], F32)
    egrep = inp("egrep", [128, D], F32)
    ebrep = inp("ebrep", [128, D], F32)
    poskt = inp("poskt", [c.L, 128, JT, c.PW], BF16)
    posqt = inp("posqt", [c.L, 128, JT, c.PW], BF16)
    wqkv = inp("wqkv", [c.L, 128, DT, 3 * D], BF16)
    bqkv = inp("bqkv", [c.L, 128, 2 * JT], F32)
    bvrep = inp("bvrep", [c.L, 128, D], F32)
    wo = inp("wo", [c.L, 128, JT, D], BF16)
    bo2 = inp("bo2", [c.L, 1, D], BF16)
    w1 = inp("w1", [c.L, 128, DT, c.F], BF16)
    b1 = inp("b1", [c.L, 128, FT], F32)
    w2 = inp("w2", [c.L, 128, FT, D], BF16)
    b22 = inp("b22", [c.L, 1, D], BF16)
    ln1g = inp("ln1g", [c.L, 128, DT], F32)
    ln1b = inp("ln1b", [c.L, 128, DT], F32)
    ln2g = inp("ln2g", [c.L, 128, DT], F32)
    ln2b = inp("ln2b", [c.L, 128, DT], F32)

    out_hT = nc.dram_tensor("out_hT", [128, DT, S2], F32, kind="ExternalOutput")

    pairs = [[2 * i, 2 * i + 1] for i in range(c.n_cores // 2)]

    def allgather(src, dst):
        """src: [128, DT, S2] bf16 tile; dst: [2, 128, DT, S2] tile."""
        if c.n_cores == 1 or c.no_cc:
            nc.sync.dma_start(dst[0], src[:])
            nc.sync.dma_start(dst[1], src[:])
        else:
            nc.gpsimd.collective_compute(
                "AllGather", OP.bypass, replica_groups=pairs,
                ins=[src.opt()], outs=[dst.opt()])

    with tile.TileContext(nc) as tc:
        import contextlib
        est = contextlib.ExitStack()
        with est:
            const = est.enter_context(tc.tile_pool(name="const", bufs=1))
            resid = est.enter_context(tc.tile_pool(name="resid", bufs=1))
            dramp = est.enter_context(tc.tile_pool(name="dramp", bufs=6,
                                                   space="DRAM"))

            identT = const.tile([128, 128], F32)
            make_identity(nc, identT[:])
            ident8 = const.tile([128, 128], FP8)
            nc.gpsimd.memset(ident8[:], 2.0 ** -8)
            nc.gpsimd.affine_select(
                out=ident8[:], in_=ident8[:], compare_op=OP.is_equal, fill=0.0,
                base=0, pattern=[[-1, 128]], channel_multiplier=1)
            ones1x64 = const.tile([1, 64], F32)
            nc.vector.memset(ones1x64[:], 1.0)
            ones32 = const.tile([128, 1], F32)
            nc.vector.memset(ones32[:], 1.0)
            onesrow = const.tile([1, S2], BF16)
            nc.vector.memset(onesrow[:], 1.0)
            eps2 = const.tile([1, 1], F32)
            nc.vector.memset(eps2[:], float(D) ** 2 * 1e-12)
            invD_row = const.tile([1, 128], F32)
            nc.vector.memset(invD_row[:], 1.0 / D)
            D_row = const.tile([1, 128], F32)
            nc.vector.memset(D_row[:], float(D))
            mb_sb = const.tile([128, TTk], F32)
            nc.sync.dma_start(mb_sb[:], maskbias.ap())

            # persistent state; split per d-tile and per 256-token chunk
            # (dep tracking is tile-granular -- separate tiles avoid false
            # write-read serialization between chunks)
            S4 = S2 // 2
            hTbf = [resid.tile([128, DT, S2], BF16, name=f"hTbf{h}")
                    for h in range(2)]                # global h halves (bf16)
            hQ32 = [[resid.tile([128, S4], F32, name=f"hQ32_{d}_{k}")
                     for k in range(2)] for d in range(DT)]
            hQbf = [[resid.tile([128, S4], BF16, name=f"hQbf{d}_{k}")
                     for k in range(2)] for d in range(DT)]

            consts = dict(identT=identT, ident8=ident8, ones1x64=ones1x64,
                          ones32=ones32, onesrow=onesrow, eps2=eps2,
                          invD_row=invD_row, D_row=D_row, mb_sb=mb_sb)
            ins = dict(poskt=poskt, posqt=posqt, wqkv=wqkv, bqkv=bqkv,
                       bvrep=bvrep, wo=wo, bo2=bo2, w1=w1, b1=b1, w2=w2,
                       b22=b22, ln1g=ln1g, ln1b=ln1b, ln2g=ln2g, ln2b=ln2b)

            # ---------------- embedding (own half only) ----------------
            with (
                tc.tile_pool(name="embp", bufs=1) as embp,
                tc.tile_pool(name="embps", bufs=2, space="PSUM") as embps,
            ):
                ids_sb = embp.tile([128, S2 // 16], I16)
                nc.sync.dma_start(ids_sb[:], ids16.ap())
                gb = embp.tile([128, TTq, D], F32)
                nc.gpsimd.dma_gather(
                    gb[:], tok_emb.ap(), ids_sb[:], num_idxs=S2,
                    num_idxs_reg=S2, elem_size=D)

                s0 = embp.tile([128, D], F32)
                nc.sync.dma_start(s0[:], seg0rep.ap())
                sd = embp.tile([128, D], F32)
                nc.sync.dma_start(sd[:], segdrep.ap())
                ssel = embp.tile([128, TTq], F32)
                nc.sync.dma_start(ssel[:], segsel.ap())
                mt = embp.tile([128, TTq], F32)
                nc.sync.dma_start(mt[:], maskt.ap())
                eg = embp.tile([128, D], F32)
                nc.sync.dma_start(eg[:], egrep.ap())
                eb = embp.tile([128, D], F32)
                nc.sync.dma_start(eb[:], ebrep.ap())

                s0b = s0[:, None, :].to_broadcast((128, TTq, D))
                nc.vector.tensor_tensor(gb[:], gb[:], s0b, OP.add)
                for tt in range(TTq):
                    nc.vector.scalar_tensor_tensor(
                        gb[:, tt], sd[:], ssel[:, tt:tt + 1], gb[:, tt],
                        OP.mult, OP.add)

                mean = embp.tile([128, TTq, 1], F32)
                nc.vector.tensor_reduce(mean[:], gb[:], mybir.AxisListType.X,
                                        OP.add)
                nc.vector.tensor_scalar_mul(mean[:], mean[:], 1.0 / D)
                nc.vector.tensor_tensor(
                    gb[:], gb[:], mean[:].to_broadcast((128, TTq, D)),
                    OP.subtract)
                sq = embp.tile([128, TTq, D], F32)
                nc.scalar.square(sq[:], gb[:])
                var = embp.tile([128, TTq, 1], F32)
                nc.vector.tensor_reduce(var[:], sq[:], mybir.AxisListType.X,
                                        OP.add)
                nc.vector.tensor_scalar(
                    var[:], var[:], 1.0 / D, 1e-12, OP.mult, OP.add)
                rstd = embp.tile([128, TTq, 1], F32)
                nc.vector.reciprocal(rstd[:], var[:])
                nc.scalar.sqrt(rstd[:], rstd[:])
                nc.vector.tensor_tensor(
                    gb[:], gb[:], rstd[:].to_broadcast((128, TTq, D)), OP.mult)
                egb = eg[:, None, :].to_broadcast((128, TTq, D))
                nc.vector.tensor_tensor(gb[:], gb[:], egb, OP.mult)
                ebb = eb[:, None, :].to_broadcast((128, TTq, D))
                nc.vector.tensor_tensor(gb[:], gb[:], ebb, OP.add)
                for tt in range(TTq):
                    nc.vector.tensor_scalar_mul(gb[:, tt], gb[:, tt],
                                                mt[:, tt:tt + 1])

                for tt in range(TTq):
                    chk, tof = tt // 2, (tt % 2) * 128
                    for dt in range(DT):
                        pst = embps.tile([128, 128], F32, tag="tp")
                        nc.tensor.transpose(
                            pst[:], gb[:, tt, dt * 128:(dt + 1) * 128],
                            identT[:])
                        nc.scalar.copy(hQ32[dt][chk][:, tof:tof + 128],
                                       pst[:])
                        nc.vector.tensor_copy(
                            hQbf[dt][chk][:, tof:tof + 128], pst[:])

            # embedding AG
            ag_in_e = dramp.tile([128, DT, S2], BF16, tag="agi", name="agi_e")
            ag_out_e = dramp.tile([2, 128, DT, S2], BF16, tag="ago",
                                  name="ago_e")
            for dt in range(DT):
                for chk in range(2):
                    nc.sync.dma_start(
                        ag_in_e[:, dt, chk * (S2 // 2):(chk + 1) * (S2 // 2)],
                        hQbf[dt][chk][:])
            allgather(ag_in_e, ag_out_e)
            pend_ag = ag_out_e

            # ---------------- layers ----------------
            for l in range(c.L):
                pend_ag = layer(nc, tc, c, l, hTbf, hQ32, hQbf, dramp,
                                consts, ins, pend_ag, allgather)

            for dt in range(DT):
                for chk in range(2):
                    nc.sync.dma_start(
                        out_hT.ap()[:, dt,
                                    chk * (S2 // 2):(chk + 1) * (S2 // 2)],
                        hQ32[dt][chk][:])

    nc.compile()
    return nc


def _ln_local(nc, c, lp, lps, pps, pbs, x_sb, hQ32, hQbf, g_sb, b_sb,
              consts):
    """Feature-major layernorm over one token chunk, fully in SBUF.
    x_sb/hQ32/hQbf: lists of DT [128, CW] chunk tiles."""
    DT, D = c.DT, c.D
    CW = c.S2 // 2
    eps2, invD_row, D_row = consts["eps2"], consts["invD_row"], consts["D_row"]
    ones32 = consts["ones32"]

    stats0 = pps.tile([1, CW], F32, tag="s0")
    stats1 = pps.tile([1, CW], F32, tag="s1")
    for dt in range(DT):
        x2t = lp.tile([128, CW], F32, tag="lnx2")
        nc.scalar.square(x2t[:], x_sb[dt][:])
        nc.tensor.matmul(stats0[:], lhsT=ones32[:], rhs=x_sb[dt][:],
                         start=(dt == 0), stop=(dt == DT - 1))
        nc.tensor.matmul(stats1[:], lhsT=ones32[:], rhs=x2t[:],
                         start=(dt == 0), stop=(dt == DT - 1))
    s0r = lps.tile([1, CW], F32, tag="s0r")
    nc.scalar.copy(s0r[:], stats0[:])
    s1r = lps.tile([1, CW], F32, tag="s1r")
    nc.scalar.copy(s1r[:], stats1[:])
    u = lps.tile([1, CW], F32, tag="u")
    nc.vector.tensor_tensor(u[:], s0r[:], s0r[:], OP.mult)
    nc.vector.scalar_tensor_tensor(
        u[:], s1r[:], float(D), u[:], OP.mult, OP.subtract)
    nc.scalar.activation(u[:], u[:], AF.Sqrt, bias=eps2[:], scale=1.0)
    rp = lps.tile([1, CW], F32, tag="rp")
    nc.vector.reciprocal(rp[:], u[:])
    pm = pbs.tile([128, CW], F32, tag="bc")
    nc.tensor.matmul(pm[:], lhsT=invD_row[:], rhs=s0r[:], start=True, stop=True)
    mu_b = lps.tile([128, CW], F32, tag="mub")
    nc.scalar.copy(mu_b[:], pm[:])
    pr = pbs.tile([128, CW], F32, tag="bc")
    nc.tensor.matmul(pr[:], lhsT=D_row[:], rhs=rp[:], start=True, stop=True)
    rs_b = lps.tile([128, CW], F32, tag="rsb")
    nc.scalar.copy(rs_b[:], pr[:])

    for dt in range(DT):
        t = lp.tile([128, CW], F32, tag="lnt")
        nc.vector.tensor_tensor(t[:], x_sb[dt][:], mu_b[:], OP.subtract)
        nc.vector.tensor_tensor(t[:], t[:], rs_b[:], OP.mult)
        nc.vector.tensor_scalar(
            hQ32[dt][:], t[:], g_sb[:, dt:dt + 1], b_sb[:, dt:dt + 1],
            OP.mult, OP.add)
        nc.scalar.copy(hQbf[dt][:], hQ32[dt][:])


def layer(nc, tc, c, l, hTbf, hQ32, hQbf, dramp, consts, ins, pend_ag,
          allgather):
    """Emit one layer.  `pend_ag` is the [2,128,DT,S2] bf16 AG output tile
    carrying the previous LN2'd h halves; returns this layer's AG tile."""
    S, S2, D = c.S, c.S2, c.D
    DT, TTq, TTk, JT, FT = c.DT, c.TTq, c.TTk, c.JT, c.FT
    identT, ident8 = consts["identT"], consts["ident8"]
    ones1x64, onesrow, mb_sb = (consts["ones1x64"], consts["onesrow"],
                                consts["mb_sb"])
    W2Q, W2K = c.W2Q, c.W2K
    NHL = c.H  # all heads local now

    with (
        tc.tile_pool(name=f"l{l}_misc", bufs=1) as miscp,
        tc.tile_pool(name=f"l{l}_ctx", bufs=1) as ctxp,
    ):
        ctxT = [ctxp.tile([128, S2], BF16, name=f"ctxT{j}")
                for j in range(JT)]
        bq_sb = miscp.tile([128, 2 * JT], F32, name="bq_sb")
        nc.sync.dma_start(bq_sb[:], ins["bqkv"].ap()[l])
        bv_sb = miscp.tile([128, D], F32, name="bv_sb")
        nc.sync.dma_start(bv_sb[:], ins["bvrep"].ap()[l])
        bo_sb = miscp.tile([1, D], BF16, name="bo_sb")
        nc.sync.dma_start(bo_sb[:], ins["bo2"].ap()[l])

        with (
            tc.tile_pool(name=f"l{l}_qkv", bufs=1) as qkvp,
            tc.tile_pool(name=f"l{l}_pos", bufs=1) as posp,
        ):
            qsT = qkvp.tile([128, JT, S2], BF16, name="qsT")
            kT = qkvp.tile([128, JT, S], BF16, name="kT")
            v_sb = qkvp.tile([128, TTk, NHL * 65], BF16, name="v_sb")
            poskr = posp.tile([128, JT, c.PW], BF16, name="poskr")
            nc.sync.dma_start(poskr[:], ins["poskt"].ap()[l])
            posq = posp.tile([128, JT, c.PW], BF16, name="posq")
            nc.sync.dma_start(posq[:], ins["posqt"].ap()[l])

            cq_dr, ck_dr = [], []
            with (
                tc.tile_pool(name=f"l{l}_wst", bufs=4) as wstp,
                tc.tile_pool(name=f"l{l}_wvp", bufs=2) as wvp,
                tc.tile_pool(name=f"l{l}_pps", bufs=2, space="PSUM") as pps,
                tc.tile_pool(name=f"l{l}_ct", bufs=4) as ctp,
            ):
                # ---- phase A (local; overlaps incoming AG): q proj + cq
                wq_sb = wvp.tile([128, DT, D], BF16, tag="wqall")
                nc.sync.dma_start(wq_sb[:], ins["wqkv"].ap()[l, :, :, 0:D])
                for chk in range(2):
                    cs = slice(chk * (S2 // 2), (chk + 1) * (S2 // 2))
                    for jt in range(JT):
                        ps = pps.tile([128, S2 // 2], F32, tag="qkv")
                        mm_accl(nc, ps[:],
                                wq_sb[:, :, jt * 128:(jt + 1) * 128],
                                [h[chk][:] for h in hQbf], DT, True, True)
                        nc.scalar.activation(
                            qsT[:, jt, cs], ps[:], AF.Identity,
                            bias=bq_sb[:, jt:jt + 1], scale=c.scale)

                for hl in range(NHL):
                    jt, rb = hl // 2, 64 * (hl % 2)
                    cq = dramp.tile([S2, W2Q], FP8, tag="cq",
                                    name=f"cq{l}_{hl}")
                    cq_dr.append(cq)
                    qh = qsT[rb:rb + 64, jt]
                    pkh = poskr[rb:rb + 64, jt]
                    th, base = cq[:].tensor, cq[:].offset
                    for rt in range(TTq):
                        st = ctp.tile([128, W2Q], FP8, tag="cstage")
                        off = 384 - rt * 128
                        for co in range(0, W2Q, 512):
                            w = min(512, W2Q - co)
                            ps = pps.tile([128, 512], F32, tag="ctab")
                            nc.tensor.matmul(
                                ps[:, :w],
                                lhsT=qh[:, rt * 128:(rt + 1) * 128],
                                rhs=pkh[:, off + co:off + co + w],
                                start=True, stop=True)
                            nc.scalar.activation(
                                st[:, co:co + w], ps[:, :w], AF.Copy,
                                scale=256.0)
                        dst = bass.AP(th, base + (rt * 128) * W2Q,
                                      [[W2Q, 128], [1, W2Q]])
                        nc.sync.dma_start(dst, st[:])

                # ---- phase B: consume AG -> hTbf; k/v proj; ck tables ----
                nc.sync.dma_start(hTbf[0][:], pend_ag[0])
                nc.sync.dma_start(hTbf[1][:], pend_ag[1])

                for jt in range(JT):
                    wt = wstp.tile([128, DT, 128], BF16, tag="wk")
                    nc.sync.dma_start(
                        wt[:], ins["wqkv"].ap()[l, :, :,
                                                D + jt * 128:
                                                D + (jt + 1) * 128])
                    for ch in range(2):
                        ps = pps.tile([128, S2], F32, tag="qkv")
                        mm_acc(nc, ps[:], wt[:], hTbf[ch][:],
                               DT, True, True)
                        nc.scalar.activation(
                            kT[:, jt, ch * S2:(ch + 1) * S2], ps[:],
                            AF.Identity,
                            bias=bq_sb[:, JT + jt:JT + jt + 1], scale=1.0)

                for half in range(2):
                    wt = wvp.tile([128, DT, 512], BF16, tag="wv")
                    nc.sync.dma_start(
                        wt[:], ins["wqkv"].ap()[l, :, :,
                                                2 * D + half * 512:
                                                2 * D + (half + 1) * 512])
                    for tt in range(TTk):
                        tch, tof = tt // 4, (tt % 4) * 128
                        ps = pps.tile([128, 512], F32, tag="vproj")
                        mm_acc(nc, ps[:],
                               hTbf[tch][:, :, tof:tof + 128],
                               wt[:], DT, True, True)
                        for hh in range(8):
                            hl = half * 8 + hh
                            nc.vector.tensor_tensor(
                                v_sb[:, tt, hl * 65:hl * 65 + 64],
                                ps[:, hh * 64:(hh + 1) * 64],
                                bv_sb[:, hl * 64:hl * 64 + 64], OP.add)
                for hl in range(NHL):
                    nc.vector.memset(
                        v_sb[:, :, hl * 65 + 64:hl * 65 + 65], 1.0)

            # ---- phase C: per-head attention (ck build interleaved) ----
            with (
                tc.tile_pool(name=f"l{l}_ctk", bufs=4) as ctkp,
                tc.tile_pool(name=f"l{l}_g1", bufs=2) as g1p,
                tc.tile_pool(name=f"l{l}_g2", bufs=3) as g2p,
                tc.tile_pool(name=f"l{l}_ex", bufs=2) as exp_,
                tc.tile_pool(name=f"l{l}_sc", bufs=2) as scp,
                tc.tile_pool(name=f"l{l}_bps", bufs=2, space="PSUM") as bps,
                tc.tile_pool(name=f"l{l}_bsc", bufs=3, space="PSUM") as bsc,
                tc.tile_pool(name=f"l{l}_bp2", bufs=2, space="PSUM") as bps2,
                tc.tile_pool(name=f"l{l}_bp3", bufs=1, space="PSUM") as bps3,
            ):
                for hl in range(NHL):
                    jt, rb = hl // 2, 64 * (hl % 2)
                    qh = qsT[rb:rb + 64, jt]
                    kh = kT[rb:rb + 64, jt]

                    # build ck (p2c) table for this head
                    ck = dramp.tile([S, W2K], FP8, tag="ck",
                                    name=f"ck{l}_{hl}")
                    ck_dr.append(ck)
                    pqh = posq[rb:rb + 64, jt]
                    th, base = ck[:].tensor, ck[:].offset
                    for rt in range(TTk):
                        st = ctkp.tile([128, W2K], FP8, tag="kstage")
                        off = 896 - rt * 128
                        for co in range(0, W2K, 512):
                            w = min(512, W2K - co)
                            ps = bps.tile([128, 512], F32, tag="ctab")
                            nc.tensor.matmul(
                                ps[:, :w],
                                lhsT=kh[:, rt * 128:(rt + 1) * 128],
                                rhs=pqh[:, off + co:off + co + w],
                                start=True, stop=True)
                            nc.vector.tensor_scalar_mul(
                                st[:, co:co + w], ps[:, :w], 256.0)
                        dst = bass.AP(th, base + (rt * 128) * W2K,
                                      [[W2K, 128], [1, W2K]])
                        nc.sync.dma_start(dst, st[:])

                    g1 = g1p.tile([128, TTq, S], FP8, tag="g1")
                    thq, bq_ = cq_dr[hl][:].tensor, cq_dr[hl][:].offset
                    for qt in range(TTq):
                        src = bass.AP(thq, bq_ + W2Q * (qt * 128) + 127,
                                      [[W2Q - 1, 128], [1, S]])
                        nc.sync.dma_start(g1[:, qt], src)

                    ex = exp_.tile([128, TTk, S2], BF16, tag="ex")
                    thk, bk_ = ck[:].tensor, ck[:].offset
                    for kt in range(TTk):
                        g2 = g2p.tile([128, S2], FP8, tag="g2",
                                      name=f"g2_{kt}")
                        src = bass.AP(thk, bk_ + W2K * (kt * 128) + 127,
                                      [[W2K - 1, 128], [1, S2]])
                        nc.sync.dma_start(g2[:], src)
                        ps = bsc.tile([128, S2], F32, tag="scores")
                        nc.tensor.matmul(
                            ps[:], lhsT=kh[:, kt * 128:(kt + 1) * 128],
                            rhs=qh[:], start=True, stop=False)
                        nc.tensor.matmul(
                            ps[:], lhsT=ident8[:], rhs=g2[:],
                            start=False, stop=False)
                        for qi in range(TTq):
                            nc.tensor.matmul(
                                ps[:, qi * 128:(qi + 1) * 128],
                                lhsT=g1[:, qi, kt * 128:(kt + 1) * 128],
                                rhs=ident8[:],
                                start=False, stop=True,
                                skip_group_check=(qi != TTq - 1))
                        nc.scalar.activation(
                            ex[:, kt], ps[:], AF.Exp,
                            bias=mb_sb[:, kt:kt + 1], scale=1.0)

                    pv = bps2.tile([65, S2], F32, tag="pv")
                    for kt in range(TTk):
                        nc.tensor.matmul(
                            pv[:], lhsT=v_sb[:, kt, hl * 65:hl * 65 + 65],
                            rhs=ex[:, kt],
                            start=(kt == 0), stop=(kt == TTk - 1))
                    rec = scp.tile([1, S2], F32, tag="rec")
                    nc.vector.reciprocal(rec[:], pv[64:65, :])
                    pb = bps3.tile([64, S2], F32, tag="recb")
                    nc.tensor.matmul(pb[:], lhsT=ones1x64[:], rhs=rec[:],
                                     start=True, stop=True)
                    rb_sb = scp.tile([64, S2], F32, tag="recbs")
                    nc.scalar.copy(rb_sb[:], pb[:])
                    nc.vector.tensor_tensor(
                        ctxT[jt][rb:rb + 64], pv[0:64, :], rb_sb[:], OP.mult)

        # ---- phase D: Wo + residual + LN1 (all local) ----
        with (
            tc.tile_pool(name=f"l{l}_wops", bufs=3, space="PSUM") as wops,
            tc.tile_pool(name=f"l{l}_wo", bufs=1) as wopool,
            tc.tile_pool(name=f"l{l}_xa", bufs=1) as xap,
            tc.tile_pool(name=f"l{l}_lnp", bufs=2) as lnp,
            tc.tile_pool(name=f"l{l}_lns", bufs=1) as lns,
            tc.tile_pool(name=f"l{l}_lnps", bufs=1, space="PSUM") as lnps,
            tc.tile_pool(name=f"l{l}_lnpb", bufs=2, space="PSUM") as lnpb,
        ):
            wos = wopool.tile([128, JT, D], BF16, tag="wo")
            nc.sync.dma_start(wos[:], ins["wo"].ap()[l])
            g1_sb = lns.tile([128, DT], F32, name="g1_sb")
            nc.sync.dma_start(g1_sb[:], ins["ln1g"].ap()[l])
            bn1_sb = lns.tile([128, DT], F32, name="bn1_sb")
            nc.sync.dma_start(bn1_sb[:], ins["ln1b"].ap()[l])

            xa = [[xap.tile([128, S2 // 2], F32, name=f"xa{dt}_{k}")
                   for k in range(2)] for dt in range(DT)]
            for chk in range(2):
                cs = slice(chk * (S2 // 2), (chk + 1) * (S2 // 2))
                for dt in range(DT):
                    ps = wops.tile([128, S2 // 2], F32, tag="wo")
                    mm_accl(nc, ps[:], wos[:, :, dt * 128:(dt + 1) * 128],
                            [t[:, cs] for t in ctxT], JT, True, False)
                    nc.tensor.matmul(
                        ps[:], lhsT=bo_sb[:, dt * 128:(dt + 1) * 128],
                        rhs=onesrow[:, cs], start=False, stop=True)
                    nc.vector.tensor_tensor(xa[dt][chk][:],
                                            hQ32[dt][chk][:], ps[:], OP.add)
            for chk in range(2):
                _ln_local(nc, c, lnp, lns, lnps, lnpb,
                          [xa[dt][chk] for dt in range(DT)],
                          [hQ32[dt][chk] for dt in range(DT)],
                          [hQbf[dt][chk] for dt in range(DT)],
                          g1_sb, bn1_sb, consts)

    # ---- phase E: FFN + LN2 + AG ----
    with (
        tc.tile_pool(name=f"l{l}_dmisc", bufs=1) as dmiscp,
        tc.tile_pool(name=f"l{l}_gt", bufs=1) as gtp,
        tc.tile_pool(name=f"l{l}_w1", bufs=4) as w1pool,
        tc.tile_pool(name=f"l{l}_w2", bufs=1) as w2pool,
        tc.tile_pool(name=f"l{l}_f1ps", bufs=2, space="PSUM") as f1ps,
        tc.tile_pool(name=f"l{l}_f2ps", bufs=2, space="PSUM") as f2ps,
        tc.tile_pool(name=f"l{l}_xb", bufs=1) as xbp,
        tc.tile_pool(name=f"l{l}_elnp", bufs=2) as elnp,
        tc.tile_pool(name=f"l{l}_elns", bufs=1) as elns,
        tc.tile_pool(name=f"l{l}_elnps", bufs=1, space="PSUM") as elnps,
        tc.tile_pool(name=f"l{l}_elnpb", bufs=2, space="PSUM") as elnpb,
    ):
        b1_sb2 = dmiscp.tile([128, FT], F32, name="b1_sb2")
        nc.sync.dma_start(b1_sb2[:], ins["b1"].ap()[l])
        b2_sb2 = dmiscp.tile([1, D], BF16, name="b2_sb2")
        nc.sync.dma_start(b2_sb2[:], ins["b22"].ap()[l])
        g2_sb = elns.tile([128, DT], F32, name="g2_sb")
        nc.sync.dma_start(g2_sb[:], ins["ln2g"].ap()[l])
        bn2_sb = elns.tile([128, DT], F32, name="bn2_sb")
        nc.sync.dma_start(bn2_sb[:], ins["ln2b"].ap()[l])
        w2s = w2pool.tile([128, FT, D], BF16, name="w2s")
        nc.sync.dma_start(w2s[:], ins["w2"].ap()[l])

        gt = [gtp.tile([128, FT, S2 // 2], BF16, name=f"gt{k}")
              for k in range(2)]
        xb = [[xbp.tile([128, S2 // 2], F32, name=f"xb{dt}_{k}")
               for k in range(2)] for dt in range(DT)]
        for chk in range(2):
            cs = slice(chk * (S2 // 2), (chk + 1) * (S2 // 2))
            for ft in range(FT):
                wt = w1pool.tile([128, DT, 128], BF16, tag="w1t")
                nc.sync.dma_start(
                    wt[:], ins["w1"].ap()[l, :, :, ft * 128:(ft + 1) * 128])
                ps = f1ps.tile([128, S2 // 2], F32, tag="f1")
                mm_accl(nc, ps[:], wt[:], [h[chk][:] for h in hQbf], DT,
                        True, True)
                nc.scalar.activation(
                    gt[chk][:, ft], ps[:],
                    AF.Gelu if c.act == "gelu" else AF.Relu,
                    bias=b1_sb2[:, ft:ft + 1], scale=1.0)
        for chk in range(2):
            cs = slice(chk * (S2 // 2), (chk + 1) * (S2 // 2))
            for dt in range(DT):
                ps = f2ps.tile([128, S2 // 2], F32, tag="f2")
                mm_acc(nc, ps[:], w2s[:, :, dt * 128:(dt + 1) * 128],
                       gt[chk][:], FT, True, False)
                nc.tensor.matmul(
                    ps[:], lhsT=b2_sb2[:, dt * 128:(dt + 1) * 128],
                    rhs=onesrow[:, cs], start=False, stop=True)
                nc.vector.tensor_tensor(xb[dt][chk][:], hQ32[dt][chk][:],
                                        ps[:], OP.add)
        for chk in range(2):
            _ln_local(nc, c, elnp, elns, elnps, elnpb,
                      [xb[dt][chk] for dt in range(DT)],
                      [hQ32[dt][chk] for dt in range(DT)],
                      [hQbf[dt][chk] for dt in range(DT)],
                      g2_sb, bn2_sb, consts)

        if l < c.L - 1:
            ag_in = dramp.tile([128, c.DT, S2], BF16, tag="agi",
                               name=f"agi_{l}")
            ag_out = dramp.tile([2, 128, c.DT, S2], BF16, tag="ago",
                                name=f"ago_{l}")
            for dt in range(c.DT):
                for chk in range(2):
                    nc.sync.dma_start(
                        ag_in[:, dt, chk * (S2 // 2):(chk + 1) * (S2 // 2)],
                        hQbf[dt][chk][:])
            allgather(ag_in, ag_out)
            return ag_out
    return None


# ---------------------------------------------------------------------------
# host side
# ---------------------------------------------------------------------------

def host_prep(c, inputs):
    """Build per-core in_maps from full inputs."""
    bf = ml_dtypes.bfloat16
    f32 = np.float32
    ii = {k: np.asarray(v) for k, v in inputs.items()}
    S, S2, D, L = c.S, c.S2, c.D, c.L

    def tokmaj(vec, nt):  # [nt*128] -> [128, nt]
        return np.ascontiguousarray(vec.reshape(nt, 128).T)

    rel = ii["rel_emb"].astype(f32)  # [2*SPAN, D]

    # full-weight program tensors (rank-independent): build once
    wq_f = ii["Wq"].astype(f32)                       # [L, D, D]
    wk_f = ii["Wk"].astype(f32)
    wv_f = ii["Wv"].astype(f32)
    wqkv = np.concatenate([wq_f, wk_f, wv_f], axis=2)  # [L, D, 3D]
    wqkv = wqkv.reshape(L, c.DT, 128, 3 * D).transpose(0, 2, 1, 3)
    wqkv = np.ascontiguousarray(wqkv.astype(bf))

    bq = ii["bq"].astype(f32) * c.scale               # [L, D]
    bk = ii["bk"].astype(f32)
    bqkv = np.concatenate(
        [bq.reshape(L, c.JT, 128).transpose(0, 2, 1),
         bk.reshape(L, c.JT, 128).transpose(0, 2, 1)], axis=2)
    bqkv = np.ascontiguousarray(bqkv)
    bvrep = np.ascontiguousarray(np.broadcast_to(
        ii["bv"].astype(f32)[:, None, :], (L, 128, D)))

    wo_ = ii["Wo"].astype(f32).reshape(L, c.JT, 128, D).transpose(0, 2, 1, 3)
    wo_ = np.ascontiguousarray(wo_.astype(bf))
    bo2 = np.ascontiguousarray(
        ii["bo"].astype(f32)[:, None, :].astype(bf))

    w1_ = ii["W1"].astype(f32).reshape(L, c.DT, 128, c.F).transpose(0, 2, 1, 3)
    w1_ = np.ascontiguousarray(w1_.astype(bf))
    b1_ = np.ascontiguousarray(
        ii["b1"].astype(f32).reshape(L, c.FT, 128).transpose(0, 2, 1))
    w2_ = ii["W2"].astype(f32).reshape(L, c.FT, 128, D).transpose(0, 2, 1, 3)
    w2_ = np.ascontiguousarray(w2_.astype(bf))
    b22 = np.ascontiguousarray(
        ii["b2"].astype(f32)[:, None, :].astype(bf))

    lns = {
        k: np.ascontiguousarray(
            ii[k2].astype(f32).reshape(L, c.DT, 128).transpose(0, 2, 1))
        for k, k2 in (("ln1g", "ln1_g"), ("ln1b", "ln1_b"),
                      ("ln2g", "ln2_g"), ("ln2b", "ln2_b"))
    }

    # per-rank pos tables
    pos_tabs = {}
    for r in range(2):
        poskt = np.zeros((L, 128, c.JT, c.PW), f32)
        posqt = np.zeros((L, 128, c.JT, c.PW), f32)
        idx_k = np.clip(1023 + 512 * r - np.arange(c.PW), 0, 2 * c.SPAN - 1)
        idx_q = np.clip(np.arange(c.PW) - 511 + 512 * r, 0, 2 * c.SPAN - 1)
        for l in range(L):
            pk = rel @ wk_f[l] + ii["bk"][l].astype(f32)          # [1024, D]
            pq = (rel @ wq_f[l] + ii["bq"][l].astype(f32)) * c.scale
            for tab, idx, dst in ((pk, idx_k, poskt), (pq, idx_q, posqt)):
                ext = tab[idx]                                    # [PW, D]
                dst[l] = ext.T.reshape(c.JT, 128, c.PW).transpose(1, 0, 2)
        pos_tabs[r] = (np.ascontiguousarray(poskt.astype(bf)),
                       np.ascontiguousarray(posqt.astype(bf)))

    tok_emb_f = np.ascontiguousarray(ii["tok_emb"].astype(f32))
    seg0rep = np.ascontiguousarray(
        np.broadcast_to(ii["seg_emb"][0].astype(f32), (128, D)))
    segdrep = np.ascontiguousarray(np.broadcast_to(
        (ii["seg_emb"][1] - ii["seg_emb"][0]).astype(f32), (128, D)))
    egrep = np.ascontiguousarray(
        np.broadcast_to(ii["emb_ln_g"].astype(f32), (128, D)))
    ebrep = np.ascontiguousarray(
        np.broadcast_to(ii["emb_ln_b"].astype(f32), (128, D)))

    in_maps = []
    for core in range(c.n_cores):
        b, r = core // 2, core % 2
        tsl = slice(r * S2, (r + 1) * S2)

        ids = ii["input_ids"][b, tsl].astype(np.int64)
        w = np.zeros((16, S2 // 16), np.int16)
        for i in range(S2):
            w[i % 16, i // 16] = ids[i]
        ids16 = np.tile(w, (8, 1))

        seg = ii["segment_ids"][b].astype(f32)
        mask = ii["attention_mask"][b].astype(f32)

        m = {
            "ids16": ids16,
            "tok_emb": tok_emb_f,
            "segsel": tokmaj(seg[tsl], c.TTq),
            "seg0rep": seg0rep,
            "segdrep": segdrep,
            "maskt": tokmaj(mask[tsl], c.TTq),
            "maskbias": tokmaj(NEG * (1.0 - mask), c.TTk),
            "egrep": egrep,
            "ebrep": ebrep,
            "poskt": pos_tabs[r][0],
            "posqt": pos_tabs[r][1],
            "wqkv": wqkv,
            "bqkv": bqkv,
            "bvrep": bvrep,
            "wo": wo_,
            "bo2": bo2,
            "w1": w1_,
            "b1": b1_,
            "w2": w2_,
            "b22": b22,
            **lns,
        }
        in_maps.append(m)
    return in_maps


def assemble(c, results):
    """results[core]["out_hT"] [128, DT, S2] -> [B, S, D] fp32."""
    out = np.zeros((c.B, c.S, c.D), np.float32)
    for b in range(c.B):
        for r in range(2):
            hT = results[2 * b + r]["out_hT"]  # [128, DT, S2]
            out[b, r * c.S2:(r + 1) * c.S2] = (
                hT.transpose(2, 1, 0).reshape(c.S2, c.D))
    return out


_nc_cache = {}


def _get_nc(c):
    key = (c.B, c.S, c.D, c.H, c.F, c.L, c.V, c.SPAN, c.n_cores, c.no_cc)
    if key not in _nc_cache:
        _nc_cache[key] = build_nc(c)
    return _nc_cache[key]


def kernel(**inputs):
    from concourse import bass_utils
    c = Cfg()
    nc = _get_nc(c)
    in_maps = host_prep(c, inputs)
    res = bass_utils.run_bass_kernel_spmd(
        nc, in_maps, core_ids=list(range(c.n_cores)))
    return assemble(c, res.results)


# revision 39
# speedup vs baseline: 1.1389x; 1.1389x over previous
"""DeBERTa-bare Trainium2 Bass kernel, v2: sequence-parallel TP.

Topology: 8 NeuronCores = 4 data-parallel pairs (one batch element each) x
2-way sequence parallel.  Each core owns HALF the tokens (rank r of the
pair owns global tokens [r*512, r*512+512)) and computes ALL 16 heads and
the FULL FFN for its token half.  Attention needs K/V over the full
sequence, so the only cross-core exchange is ONE bf16 AllGather of the
post-LN2 hidden state per layer (2 MB payload) -- no AllReduce, no
partial-sum f32 dance, and LayerNorm is fully local (no DRAM bounce).

The program is identical on every core (SPMD); all rank dependence lives
in host-prepared inputs: own-half ids/segment/mask, and the clamp-extended
relative-position projection tables with the rank's +-512 token offset
baked in:
    poskt[v] = pos_k[clip(1023 + 512 r - v)]     (c2p, q-side rows)
    posqt[w] = pos_q[clip(w - 511 + 512 r)]      (p2c, k-side rows)

DeBERTa's disentangled-attention gathers are realized as affine "skew"
access-pattern DMA reads from DRAM-resident fp8 (x256) tables, injected
into the score PSUM via scaled-identity matmuls (same mechanism as v1,
re-derived for the q-half/k-full tile geometry: cq is [512, 1152] per
head, ck is [1024, 640] per head).
"""

import sys

for _p in ("/opt/trn_rl_repo",):
    if _p not in sys.path:
        sys.path.insert(0, _p)

import numpy as np
import ml_dtypes

import concourse.bass as bass
import concourse.bacc as bacc
import concourse.tile as tile
import concourse.mybir as mybir
from concourse.masks import make_identity

F32 = mybir.dt.float32
BF16 = mybir.dt.bfloat16
FP8 = mybir.dt.float8e4
I16 = mybir.dt.int16

AF = mybir.ActivationFunctionType
OP = mybir.AluOpType

NEG = -1e9


def mm_acc(nc, ps, lhsT3, rhs3, nsub, start, stop):
    """Accumulating matmul over `nsub` 128-contraction subtiles.
    lhsT3/rhs3: APs shaped [128, nsub, *]."""
    for s in range(nsub):
        nc.tensor.matmul(ps, lhsT3[:, s], rhs3[:, s],
                         start=(start and s == 0), stop=(stop and s == nsub - 1))


def mm_accl(nc, ps, lhsT3, rhs_list, nsub, start, stop):
    """mm_acc with the rhs given as a list of per-subtile [128, N] APs."""
    for s in range(nsub):
        nc.tensor.matmul(ps, lhsT3[:, s], rhs_list[s],
                         start=(start and s == 0), stop=(stop and s == nsub - 1))


class Cfg:
    def __init__(self, B=4, S=1024, D=1024, H=16, F=4096, L=4, V=32000, SPAN=512,
                 n_cores=8, act="gelu", no_cc=False):
        self.B, self.S, self.D, self.H, self.F, self.L, self.V, self.SPAN = (
            B, S, D, H, F, L, V, SPAN)
        self.n_cores = n_cores
        self.DH = D // H
        assert self.DH == 64
        self.S2 = S // 2            # own token half
        self.DT = D // 128          # d tiles
        self.TTq = self.S2 // 128   # own token tiles
        self.TTk = S // 128         # global token tiles
        self.JT = D // 128          # head-dim col tiles (2 heads per tile)
        self.FT = F // 128          # ffn col tiles (full F)
        self.W2Q = S + 128          # cq skew table width
        self.W2K = self.S2 + 128    # ck skew table width
        self.PW = 1536              # host pos tables, clamp-extended
        self.scale = 1.0 / np.sqrt(3.0 * self.DH)
        self.act = act
        self.no_cc = no_cc
        # kept for test.py compat (unused in v2)
        self.ar_chunked = False
        self.ar_f32 = True


def build_nc(cfg):
    c = cfg
    S, S2, D = c.S, c.S2, c.D
    DT, TTq, TTk, JT, FT = c.DT, c.TTq, c.TTk, c.JT, c.FT
    nc = bacc.Bacc("TRN2", target_bir_lowering=False, debug=False,
                   num_devices=c.n_cores)

    def inp(name, shape, dt):
        return nc.dram_tensor(name, list(shape), dt, kind="ExternalInput")

    ids16 = inp("ids16", [128, S2 // 16], I16)
    tok_emb = inp("tok_emb", [c.V, D], F32)
    segsel = inp("segsel", [128, TTq], F32)
    seg0rep = inp("seg0rep", [128, D], F32)
    segdrep = inp("segdrep", [128, D], F32)
    maskt = inp("maskt", [128, TTq], F32)
    maskbias = inp("maskbias", [128, TTk], F32)
    egrep = inp("egrep", [128, D], F32)
    ebrep = inp("ebrep", [128, D], F32)
    poskt = inp("poskt", [c.L, 128, JT, c.PW], BF16)
    posqt = inp("posqt", [c.L, 128, JT, c.PW], BF16)
    wqkv = inp("wqkv", [c.L, 128, DT, 3 * D], BF16)
    bqkv = inp("bqkv", [c.L, 128, 2 * JT], F32)
    bvrep = inp("bvrep", [c.L, 128, D], F32)
    wo = inp("wo", [c.L, 128, JT, D], BF16)
    bo2 = inp("bo2", [c.L, 1, D], BF16)
    w1 = inp("w1", [c.L, 128, DT, c.F], BF16)
    b1 = inp("b1", [c.L, 128, FT], F32)
    w2 = inp("w2", [c.L, 128, FT, D], BF16)
    b22 = inp("b22", [c.L, 1, D], BF16)
    ln1g = inp("ln1g", [c.L, 128, DT], F32)
    ln1b = inp("ln1b", [c.L, 128, DT], F32)
    ln2g = inp("ln2g", [c.L, 128, DT], F32)
    ln2b = inp("ln2b", [c.L, 128, DT], F32)

    out_hT = nc.dram_tensor("out_hT", [128, DT, S2], F32, kind="ExternalOutput")

    pairs = [[2 * i, 2 * i + 1] for i in range(c.n_cores // 2)]

    def allgather(src, dst):
        """src: [128, DT, S2] bf16 tile; dst: [2, 128, DT, S2] tile."""
        if c.n_cores == 1 or c.no_cc:
            nc.sync.dma_start(dst[0], src[:])
            nc.sync.dma_start(dst[1], src[:])
        else:
            nc.gpsimd.collective_compute(
                "AllGather", OP.bypass, replica_groups=pairs,
                ins=[src.opt()], outs=[dst.opt()])

    with tile.TileContext(nc) as tc:
        import contextlib
        est = contextlib.ExitStack()
        with est:
            const = est.enter_context(tc.tile_pool(name="const", bufs=1))
            resid = est.enter_context(tc.tile_pool(name="resid", bufs=1))
            dramp = est.enter_context(tc.tile_pool(name="dramp", bufs=6,
                                                   space="DRAM"))

            identT = const.tile([128, 128], F32)
            make_identity(nc, identT[:])
            ident8 = const.tile([128, 128], FP8)
            nc.gpsimd.memset(ident8[:], 2.0 ** -8)
            nc.gpsimd.affine_select(
                out=ident8[:], in_=ident8[:], compare_op=OP.is_equal, fill=0.0,
                base=0, pattern=[[-1, 128]], channel_multiplier=1)
            ones1x64 = const.tile([1, 64], F32)
            nc.vector.memset(ones1x64[:], 1.0)
            ones32 = const.tile([128, 1], F32)
            nc.vector.memset(ones32[:], 1.0)
            onesrow = const.tile([1, S2], BF16)
            nc.vector.memset(onesrow[:], 1.0)
            eps2 = const.tile([1, 1], F32)
            nc.vector.memset(eps2[:], float(D) ** 2 * 1e-12)
            invD_row = const.tile([1, 128], F32)
            nc.vector.memset(invD_row[:], 1.0 / D)
            D_row = const.tile([1, 128], F32)
            nc.vector.memset(D_row[:], float(D))
            mb_sb = const.tile([128, TTk], F32)
            nc.sync.dma_start(mb_sb[:], maskbias.ap())

            # persistent state; split per d-tile and per 256-token chunk
            # (dep tracking is tile-granular -- separate tiles avoid false
            # write-read serialization between chunks)
            S4 = S2 // 2
            hTbf = [resid.tile([128, DT, S2], BF16, name=f"hTbf{h}")
                    for h in range(2)]                # global h halves (bf16)
            hQ32 = [[resid.tile([128, S4], F32, name=f"hQ32_{d}_{k}")
                     for k in range(2)] for d in range(DT)]
            hQbf = [[resid.tile([128, S4], BF16, name=f"hQbf{d}_{k}")
                     for k in range(2)] for d in range(DT)]

            consts = dict(identT=identT, ident8=ident8, ones1x64=ones1x64,
                          ones32=ones32, onesrow=onesrow, eps2=eps2,
                          invD_row=invD_row, D_row=D_row, mb_sb=mb_sb)
            ins = dict(poskt=poskt, posqt=posqt, wqkv=wqkv, bqkv=bqkv,
                       bvrep=bvrep, wo=wo, bo2=bo2, w1=w1, b1=b1, w2=w2,
                       b22=b22, ln1g=ln1g, ln1b=ln1b, ln2g=ln2g, ln2b=ln2b)

            # ---------------- embedding (own half only) ----------------
            with (
                tc.tile_pool(name="embp", bufs=1) as embp,
                tc.tile_pool(name="embps", bufs=2, space="PSUM") as embps,
            ):
                ids_sb = embp.tile([128, S2 // 16], I16)
                nc.sync.dma_start(ids_sb[:], ids16.ap())
                gb = embp.tile([128, TTq, D], F32)
                nc.gpsimd.dma_gather(
                    gb[:], tok_emb.ap(), ids_sb[:], num_idxs=S2,
                    num_idxs_reg=S2, elem_size=D)

                s0 = embp.tile([128, D], F32)
                nc.sync.dma_start(s0[:], seg0rep.ap())
                sd = embp.tile([128, D], F32)
                nc.sync.dma_start(sd[:], segdrep.ap())
                ssel = embp.tile([128, TTq], F32)
                nc.sync.dma_start(ssel[:], segsel.ap())
                mt = embp.tile([128, TTq], F32)
                nc.sync.dma_start(mt[:], maskt.ap())
                eg = embp.tile([128, D], F32)
                nc.sync.dma_start(eg[:], egrep.ap())
                eb = embp.tile([128, D], F32)
                nc.sync.dma_start(eb[:], ebrep.ap())

                s0b = s0[:, None, :].to_broadcast((128, TTq, D))
                nc.vector.tensor_tensor(gb[:], gb[:], s0b, OP.add)
                for tt in range(TTq):
                    nc.vector.scalar_tensor_tensor(
                        gb[:, tt], sd[:], ssel[:, tt:tt + 1], gb[:, tt],
                        OP.mult, OP.add)

                mean = embp.tile([128, TTq, 1], F32)
                nc.vector.tensor_reduce(mean[:], gb[:], mybir.AxisListType.X,
                                        OP.add)
                nc.vector.tensor_scalar_mul(mean[:], mean[:], 1.0 / D)
                nc.vector.tensor_tensor(
                    gb[:], gb[:], mean[:].to_broadcast((128, TTq, D)),
                    OP.subtract)
                sq = embp.tile([128, TTq, D], F32)
                nc.scalar.square(sq[:], gb[:])
                var = embp.tile([128, TTq, 1], F32)
                nc.vector.tensor_reduce(var[:], sq[:], mybir.AxisListType.X,
                                        OP.add)
                nc.vector.tensor_scalar(
                    var[:], var[:], 1.0 / D, 1e-12, OP.mult, OP.add)
                rstd = embp.tile([128, TTq, 1], F32)
                nc.vector.reciprocal(rstd[:], var[:])
                nc.scalar.sqrt(rstd[:], rstd[:])
                nc.vector.tensor_tensor(
                    gb[:], gb[:], rstd[:].to_broadcast((128, TTq, D)), OP.mult)
                egb = eg[:, None, :].to_broadcast((128, TTq, D))
                nc.vector.tensor_tensor(gb[:], gb[:], egb, OP.mult)
                ebb = eb[:, None, :].to_broadcast((128, TTq, D))
                nc.vector.tensor_tensor(gb[:], gb[:], ebb, OP.add)
                for tt in range(TTq):
                    nc.vector.tensor_scalar_mul(gb[:, tt], gb[:, tt],
                                                mt[:, tt:tt + 1])

                for tt in range(TTq):
                    chk, tof = tt // 2, (tt % 2) * 128
                    for dt in range(DT):
                        pst = embps.tile([128, 128], F32, tag="tp")
                        nc.tensor.transpose(
                            pst[:], gb[:, tt, dt * 128:(dt + 1) * 128],
                            identT[:])
                        nc.scalar.copy(hQ32[dt][chk][:, tof:tof + 128],
                                       pst[:])
                        nc.vector.tensor_copy(
                            hQbf[dt][chk][:, tof:tof + 128], pst[:])

            # embedding AG
            ag_in_e = dramp.tile([128, DT, S2], BF16, tag="agi", name="agi_e")
            ag_out_e = dramp.tile([2, 128, DT, S2], BF16, tag="ago",
                                  name="ago_e")
            for dt in range(DT):
                for chk in range(2):
                    nc.sync.dma_start(
                        ag_in_e[:, dt, chk * (S2 // 2):(chk + 1) * (S2 // 2)],
                        hQbf[dt][chk][:])
            allgather(ag_in_e, ag_out_e)
            pend_ag = ag_out_e

            # ---------------- layers ----------------
            for l in range(c.L):
                pend_ag = layer(nc, tc, c, l, hTbf, hQ32, hQbf, dramp,
                                consts, ins, pend_ag, allgather)

            for dt in range(DT):
                for chk in range(2):
                    nc.sync.dma_start(
                        out_hT.ap()[:, dt,
                                    chk * (S2 // 2):(chk + 1) * (S2 // 2)],
                        hQ32[dt][chk][:])

    nc.compile()
    return nc


def _ln_local(nc, c, lp, lps, pps, pbs, x_sb, hQ32, hQbf, g_sb, b_sb,
              consts):
    """Feature-major layernorm over one token chunk, fully in SBUF.
    x_sb/hQ32/hQbf: lists of DT [128, CW] chunk tiles."""
    DT, D = c.DT, c.D
    CW = c.S2 // 2
    eps2, invD_row, D_row = consts["eps2"], consts["invD_row"], consts["D_row"]
    ones32 = consts["ones32"]

    stats0 = pps.tile([1, CW], F32, tag="s0")
    stats1 = pps.tile([1, CW], F32, tag="s1")
    for dt in range(DT):
        x2t = lp.tile([128, CW], F32, tag="lnx2")
        nc.scalar.square(x2t[:], x_sb[dt][:])
        nc.tensor.matmul(stats0[:], lhsT=ones32[:], rhs=x_sb[dt][:],
                         start=(dt == 0), stop=(dt == DT - 1))
        nc.tensor.matmul(stats1[:], lhsT=ones32[:], rhs=x2t[:],
                         start=(dt == 0), stop=(dt == DT - 1))
    s0r = lps.tile([1, CW], F32, tag="s0r")
    nc.scalar.copy(s0r[:], stats0[:])
    s1r = lps.tile([1, CW], F32, tag="s1r")
    nc.scalar.copy(s1r[:], stats1[:])
    u = lps.tile([1, CW], F32, tag="u")
    nc.vector.tensor_tensor(u[:], s0r[:], s0r[:], OP.mult)
    nc.vector.scalar_tensor_tensor(
        u[:], s1r[:], float(D), u[:], OP.mult, OP.subtract)
    nc.scalar.activation(u[:], u[:], AF.Sqrt, bias=eps2[:], scale=1.0)
    rp = lps.tile([1, CW], F32, tag="rp")
    nc.vector.reciprocal(rp[:], u[:])
    pm = pbs.tile([128, CW], F32, tag="bc")
    nc.tensor.matmul(pm[:], lhsT=invD_row[:], rhs=s0r[:], start=True, stop=True)
    mu_b = lps.tile([128, CW], F32, tag="mub")
    nc.scalar.copy(mu_b[:], pm[:])
    pr = pbs.tile([128, CW], F32, tag="bc")
    nc.tensor.matmul(pr[:], lhsT=D_row[:], rhs=rp[:], start=True, stop=True)
    rs_b = lps.tile([128, CW], F32, tag="rsb")
    nc.scalar.copy(rs_b[:], pr[:])

    for dt in range(DT):
        t = lp.tile([128, CW], F32, tag="lnt")
        nc.vector.tensor_tensor(t[:], x_sb[dt][:], mu_b[:], OP.subtract)
        nc.vector.tensor_tensor(t[:], t[:], rs_b[:], OP.mult)
        nc.vector.tensor_scalar(
            hQ32[dt][:], t[:], g_sb[:, dt:dt + 1], b_sb[:, dt:dt + 1],
            OP.mult, OP.add)
        nc.scalar.copy(hQbf[dt][:], hQ32[dt][:])


def layer(nc, tc, c, l, hTbf, hQ32, hQbf, dramp, consts, ins, pend_ag,
          allgather):
    """Emit one layer.  `pend_ag` is the [2,128,DT,S2] bf16 AG output tile
    carrying the previous LN2'd h halves; returns this layer's AG tile."""
    S, S2, D = c.S, c.S2, c.D
    DT, TTq, TTk, JT, FT = c.DT, c.TTq, c.TTk, c.JT, c.FT
    identT, ident8 = consts["identT"], consts["ident8"]
    ones1x64, onesrow, mb_sb = (consts["ones1x64"], consts["onesrow"],
                                consts["mb_sb"])
    W2Q, W2K = c.W2Q, c.W2K
    NHL = c.H  # all heads local now

    with (
        tc.tile_pool(name=f"l{l}_misc", bufs=1) as miscp,
        tc.tile_pool(name=f"l{l}_ctx", bufs=1) as ctxp,
    ):
        ctxT = [ctxp.tile([128, S2], BF16, name=f"ctxT{j}")
                for j in range(JT)]
        bq_sb = miscp.tile([128, 2 * JT], F32, name="bq_sb")
        nc.sync.dma_start(bq_sb[:], ins["bqkv"].ap()[l])
        bv_sb = miscp.tile([128, D], F32, name="bv_sb")
        nc.sync.dma_start(bv_sb[:], ins["bvrep"].ap()[l])
        bo_sb = miscp.tile([1, D], BF16, name="bo_sb")
        nc.sync.dma_start(bo_sb[:], ins["bo2"].ap()[l])

        with (
            tc.tile_pool(name=f"l{l}_qkv", bufs=1) as qkvp,
            tc.tile_pool(name=f"l{l}_pos", bufs=1) as posp,
        ):
            qsT = qkvp.tile([128, JT, S2], BF16, name="qsT")
            kT = qkvp.tile([128, JT, S], BF16, name="kT")
            v_sb = qkvp.tile([128, TTk, NHL * 65], BF16, name="v_sb")
            poskr = posp.tile([128, JT, c.PW], BF16, name="poskr")
            nc.sync.dma_start(poskr[:], ins["poskt"].ap()[l])
            posq = posp.tile([128, JT, c.PW], BF16, name="posq")
            nc.sync.dma_start(posq[:], ins["posqt"].ap()[l])

            cq_dr, ck_dr = [], []
            with (
                tc.tile_pool(name=f"l{l}_wst", bufs=4) as wstp,
                tc.tile_pool(name=f"l{l}_wvp", bufs=2) as wvp,
                tc.tile_pool(name=f"l{l}_pps", bufs=2, space="PSUM") as pps,
                tc.tile_pool(name=f"l{l}_ct", bufs=4) as ctp,
            ):
                # ---- phase A (local; overlaps incoming AG): q proj + cq
                wq_sb = wvp.tile([128, DT, D], BF16, tag="wqall")
                nc.sync.dma_start(wq_sb[:], ins["wqkv"].ap()[l, :, :, 0:D])
                for chk in range(2):
                    cs = slice(chk * (S2 // 2), (chk + 1) * (S2 // 2))
                    for jt in range(JT):
                        ps = pps.tile([128, S2 // 2], F32, tag="qkv")
                        mm_accl(nc, ps[:],
                                wq_sb[:, :, jt * 128:(jt + 1) * 128],
                                [h[chk][:] for h in hQbf], DT, True, True)
                        nc.scalar.activation(
                            qsT[:, jt, cs], ps[:], AF.Identity,
                            bias=bq_sb[:, jt:jt + 1], scale=c.scale)

                for hl in range(NHL):
                    jt, rb = hl // 2, 64 * (hl % 2)
                    cq = dramp.tile([S2, W2Q], FP8, tag="cq",
                                    name=f"cq{l}_{hl}")
                    cq_dr.append(cq)
                    qh = qsT[rb:rb + 64, jt]
                    pkh = poskr[rb:rb + 64, jt]
                    th, base = cq[:].tensor, cq[:].offset
                    for rt in range(TTq):
                        st = ctp.tile([128, W2Q], FP8, tag="cstage")
                        off = 384 - rt * 128
                        for co in range(0, W2Q, 512):
                            w = min(512, W2Q - co)
                            ps = pps.tile([128, 512], F32, tag="ctab")
                            nc.tensor.matmul(
                                ps[:, :w],
                                lhsT=qh[:, rt * 128:(rt + 1) * 128],
                                rhs=pkh[:, off + co:off + co + w],
                                start=True, stop=True)
                            nc.scalar.activation(
                                st[:, co:co + w], ps[:, :w], AF.Copy,
                                scale=256.0)
                        dst = bass.AP(th, base + (rt * 128) * W2Q,
                                      [[W2Q, 128], [1, W2Q]])
                        nc.sync.dma_start(dst, st[:])

                # ---- phase B: consume AG -> hTbf; k/v proj; ck tables ----
                nc.sync.dma_start(hTbf[0][:], pend_ag[0])
                nc.sync.dma_start(hTbf[1][:], pend_ag[1])

                for jt in range(JT):
                    wt = wstp.tile([128, DT, 128], BF16, tag="wk")
                    nc.sync.dma_start(
                        wt[:], ins["wqkv"].ap()[l, :, :,
                                                D + jt * 128:
                                                D + (jt + 1) * 128])
                    for ch in range(2):
                        ps = pps.tile([128, S2], F32, tag="qkv")
                        mm_acc(nc, ps[:], wt[:], hTbf[ch][:],
                               DT, True, True)
                        nc.scalar.activation(
                            kT[:, jt, ch * S2:(ch + 1) * S2], ps[:],
                            AF.Identity,
                            bias=bq_sb[:, JT + jt:JT + jt + 1], scale=1.0)

                for half in range(2):
                    wt = wvp.tile([128, DT, 512], BF16, tag="wv")
                    nc.sync.dma_start(
                        wt[:], ins["wqkv"].ap()[l, :, :,
                                                2 * D + half * 512:
                                                2 * D + (half + 1) * 512])
                    for tt in range(TTk):
                        tch, tof = tt // 4, (tt % 4) * 128
                        ps = pps.tile([128, 512], F32, tag="vproj")
                        mm_acc(nc, ps[:],
                               hTbf[tch][:, :, tof:tof + 128],
                               wt[:], DT, True, True)
                        for hh in range(8):
                            hl = half * 8 + hh
                            nc.vector.tensor_tensor(
                                v_sb[:, tt, hl * 65:hl * 65 + 64],
                                ps[:, hh * 64:(hh + 1) * 64],
                                bv_sb[:, hl * 64:hl * 64 + 64], OP.add)
                for hl in range(NHL):
                    nc.vector.memset(
                        v_sb[:, :, hl * 65 + 64:hl * 65 + 65], 1.0)

            # ---- phase C: per-head attention (ck build interleaved) ----
            with (
                tc.tile_pool(name=f"l{l}_ctk", bufs=4) as ctkp,
                tc.tile_pool(name=f"l{l}_g1", bufs=2) as g1p,
                tc.tile_pool(name=f"l{l}_g2", bufs=3) as g2p,
                tc.tile_pool(name=f"l{l}_ex", bufs=2) as exp_,
                tc.tile_pool(name=f"l{l}_sc", bufs=2) as scp,
                tc.tile_pool(name=f"l{l}_bps", bufs=2, space="PSUM") as bps,
                tc.tile_pool(name=f"l{l}_bsc", bufs=3, space="PSUM") as bsc,
                tc.tile_pool(name=f"l{l}_bp2", bufs=2, space="PSUM") as bps2,
                tc.tile_pool(name=f"l{l}_bp3", bufs=1, space="PSUM") as bps3,
            ):
                for hl in range(NHL):
                    jt, rb = hl // 2, 64 * (hl % 2)
                    qh = qsT[rb:rb + 64, jt]
                    kh = kT[rb:rb + 64, jt]

                    # build ck (p2c) table for this head
                    ck = dramp.tile([S, W2K], FP8, tag="ck",
                                    name=f"ck{l}_{hl}")
                    ck_dr.append(ck)
                    pqh = posq[rb:rb + 64, jt]
                    th, base = ck[:].tensor, ck[:].offset
                    for rt in range(TTk):
                        st = ctkp.tile([128, W2K], FP8, tag="kstage")
                        off = 896 - rt * 128
                        for co in range(0, W2K, 512):
                            w = min(512, W2K - co)
                            ps = bps.tile([128, 512], F32, tag="ctab")
                            nc.tensor.matmul(
                                ps[:, :w],
                                lhsT=kh[:, rt * 128:(rt + 1) * 128],
                                rhs=pqh[:, off + co:off + co + w],
                                start=True, stop=True)
                            nc.vector.tensor_scalar_mul(
                                st[:, co:co + w], ps[:, :w], 256.0)
                        dst = bass.AP(th, base + (rt * 128) * W2K,
                                      [[W2K, 128], [1, W2K]])
                        nc.sync.dma_start(dst, st[:])

                    g1 = g1p.tile([128, TTq, S], FP8, tag="g1")
                    thq, bq_ = cq_dr[hl][:].tensor, cq_dr[hl][:].offset
                    for qt in range(TTq):
                        src = bass.AP(thq, bq_ + W2Q * (qt * 128) + 127,
                                      [[W2Q - 1, 128], [1, S]])
                        nc.sync.dma_start(g1[:, qt], src)

                    ex = exp_.tile([128, TTk, S2], BF16, tag="ex")
                    thk, bk_ = ck[:].tensor, ck[:].offset
                    for kt in range(TTk):
                        g2 = g2p.tile([128, S2], FP8, tag="g2",
                                      name=f"g2_{kt}")
                        src = bass.AP(thk, bk_ + W2K * (kt * 128) + 127,
                                      [[W2K - 1, 128], [1, S2]])
                        nc.sync.dma_start(g2[:], src)
                        ps = bsc.tile([128, S2], F32, tag="scores")
                        nc.tensor.matmul(
                            ps[:], lhsT=kh[:, kt * 128:(kt + 1) * 128],
                            rhs=qh[:], start=True, stop=False)
                        nc.tensor.matmul(
                            ps[:], lhsT=ident8[:], rhs=g2[:],
                            start=False, stop=False)
                        for qi in range(TTq):
                            nc.tensor.matmul(
                                ps[:, qi * 128:(qi + 1) * 128],
                                lhsT=g1[:, qi, kt * 128:(kt + 1) * 128],
                                rhs=ident8[:],
                                start=False, stop=True,
                                skip_group_check=(qi != TTq - 1))
                        nc.scalar.activation(
                            ex[:, kt], ps[:], AF.Exp,
                            bias=mb_sb[:, kt:kt + 1], scale=1.0)

                    pv = bps2.tile([65, S2], F32, tag="pv")
                    for kt in range(TTk):
                        nc.tensor.matmul(
                            pv[:], lhsT=v_sb[:, kt, hl * 65:hl * 65 + 65],
                            rhs=ex[:, kt],
                            start=(kt == 0), stop=(kt == TTk - 1))
                    rec = scp.tile([1, S2], F32, tag="rec")
                    nc.vector.reciprocal(rec[:], pv[64:65, :])
                    pb = bps3.tile([64, S2], F32, tag="recb")
                    nc.tensor.matmul(pb[:], lhsT=ones1x64[:], rhs=rec[:],
                                     start=True, stop=True)
                    rb_sb = scp.tile([64, S2], F32, tag="recbs")
                    nc.scalar.copy(rb_sb[:], pb[:])
                    nc.vector.tensor_tensor(
                        ctxT[jt][rb:rb + 64], pv[0:64, :], rb_sb[:], OP.mult)

        # ---- phase D: Wo + residual + LN1 (all local) ----
        with (
            tc.tile_pool(name=f"l{l}_wops", bufs=3, space="PSUM") as wops,
            tc.tile_pool(name=f"l{l}_wo", bufs=1) as wopool,
            tc.tile_pool(name=f"l{l}_xa", bufs=1) as xap,
            tc.tile_pool(name=f"l{l}_lnp", bufs=2) as lnp,
            tc.tile_pool(name=f"l{l}_lns", bufs=1) as lns,
            tc.tile_pool(name=f"l{l}_lnps", bufs=1, space="PSUM") as lnps,
            tc.tile_pool(name=f"l{l}_lnpb", bufs=2, space="PSUM") as lnpb,
        ):
            wos = wopool.tile([128, JT, D], BF16, tag="wo")
            nc.sync.dma_start(wos[:], ins["wo"].ap()[l])
            g1_sb = lns.tile([128, DT], F32, name="g1_sb")
            nc.sync.dma_start(g1_sb[:], ins["ln1g"].ap()[l])
            bn1_sb = lns.tile([128, DT], F32, name="bn1_sb")
            nc.sync.dma_start(bn1_sb[:], ins["ln1b"].ap()[l])

            xa = [[xap.tile([128, S2 // 2], F32, name=f"xa{dt}_{k}")
                   for k in range(2)] for dt in range(DT)]
            for chk in range(2):
                cs = slice(chk * (S2 // 2), (chk + 1) * (S2 // 2))
                for dt in range(DT):
                    ps = wops.tile([128, S2 // 2], F32, tag="wo")
                    mm_accl(nc, ps[:], wos[:, :, dt * 128:(dt + 1) * 128],
                            [t[:, cs] for t in ctxT], JT, True, False)
                    nc.tensor.matmul(
                        ps[:], lhsT=bo_sb[:, dt * 128:(dt + 1) * 128],
                        rhs=onesrow[:, cs], start=False, stop=True)
                    nc.vector.tensor_tensor(xa[dt][chk][:],
                                            hQ32[dt][chk][:], ps[:], OP.add)
            for chk in range(2):
                _ln_local(nc, c, lnp, lns, lnps, lnpb,
                          [xa[dt][chk] for dt in range(DT)],
                          [hQ32[dt][chk] for dt in range(DT)],
                          [hQbf[dt][chk] for dt in range(DT)],
                          g1_sb, bn1_sb, consts)

    # ---- phase E: FFN + LN2 + AG ----
    with (
        tc.tile_pool(name=f"l{l}_dmisc", bufs=1) as dmiscp,
        tc.tile_pool(name=f"l{l}_gt", bufs=1) as gtp,
        tc.tile_pool(name=f"l{l}_w1", bufs=4) as w1pool,
        tc.tile_pool(name=f"l{l}_w2", bufs=1) as w2pool,
        tc.tile_pool(name=f"l{l}_f1ps", bufs=2, space="PSUM") as f1ps,
        tc.tile_pool(name=f"l{l}_f2ps", bufs=2, space="PSUM") as f2ps,
        tc.tile_pool(name=f"l{l}_xb", bufs=1) as xbp,
        tc.tile_pool(name=f"l{l}_elnp", bufs=2) as elnp,
        tc.tile_pool(name=f"l{l}_elns", bufs=1) as elns,
        tc.tile_pool(name=f"l{l}_elnps", bufs=1, space="PSUM") as elnps,
        tc.tile_pool(name=f"l{l}_elnpb", bufs=2, space="PSUM") as elnpb,
    ):
        b1_sb2 = dmiscp.tile([128, FT], F32, name="b1_sb2")
        nc.sync.dma_start(b1_sb2[:], ins["b1"].ap()[l])
        b2_sb2 = dmiscp.tile([1, D], BF16, name="b2_sb2")
        nc.sync.dma_start(b2_sb2[:], ins["b22"].ap()[l])
        g2_sb = elns.tile([128, DT], F32, name="g2_sb")
        nc.sync.dma_start(g2_sb[:], ins["ln2g"].ap()[l])
        bn2_sb = elns.tile([128, DT], F32, name="bn2_sb")
        nc.sync.dma_start(bn2_sb[:], ins["ln2b"].ap()[l])
        w2s = w2pool.tile([128, FT, D], BF16, name="w2s")
        nc.sync.dma_start(w2s[:], ins["w2"].ap()[l])

        gt = [gtp.tile([128, FT, S2 // 2], BF16, name=f"gt{k}")
              for k in range(2)]
        xb = [[xbp.tile([128, S2 // 2], F32, name=f"xb{dt}_{k}")
               for k in range(2)] for dt in range(DT)]
        for chk in range(2):
            cs = slice(chk * (S2 // 2), (chk + 1) * (S2 // 2))
            for ft in range(FT):
                wt = w1pool.tile([128, DT, 128], BF16, tag="w1t")
                nc.sync.dma_start(
                    wt[:], ins["w1"].ap()[l, :, :, ft * 128:(ft + 1) * 128])
                ps = f1ps.tile([128, S2 // 2], F32, tag="f1")
                mm_accl(nc, ps[:], wt[:], [h[chk][:] for h in hQbf], DT,
                        True, True)
                nc.scalar.activation(
                    gt[chk][:, ft], ps[:],
                    AF.Gelu if c.act == "gelu" else AF.Relu,
                    bias=b1_sb2[:, ft:ft + 1], scale=1.0)
        for chk in range(2):
            cs = slice(chk * (S2 // 2), (chk + 1) * (S2 // 2))
            for dt in range(DT):
                ps = f2ps.tile([128, S2 // 2], F32, tag="f2")
                mm_acc(nc, ps[:], w2s[:, :, dt * 128:(dt + 1) * 128],
                       gt[chk][:], FT, True, False)
                nc.tensor.matmul(
                    ps[:], lhsT=b2_sb2[:, dt * 128:(dt + 1) * 128],
                    rhs=onesrow[:, cs], start=False, stop=True)
                nc.vector.tensor_tensor(xb[dt][chk][:], hQ32[dt][chk][:],
                                        ps[:], OP.add)
        for chk in range(2):
            _ln_local(nc, c, elnp, elns, elnps, elnpb,
                      [xb[dt][chk] for dt in range(DT)],
                      [hQ32[dt][chk] for dt in range(DT)],
                      [hQbf[dt][chk] for dt in range(DT)],
                      g2_sb, bn2_sb, consts)

        if l < c.L - 1:
            ag_in = dramp.tile([128, c.DT, S2], BF16, tag="agi",
                               name=f"agi_{l}")
            ag_out = dramp.tile([2, 128, c.DT, S2], BF16, tag="ago",
                                name=f"ago_{l}")
            for dt in range(c.DT):
                for chk in range(2):
                    nc.sync.dma_start(
                        ag_in[:, dt, chk * (S2 // 2):(chk + 1) * (S2 // 2)],
                        hQbf[dt][chk][:])
            allgather(ag_in, ag_out)
            return ag_out
    return None


# ---------------------------------------------------------------------------
# host side
# ---------------------------------------------------------------------------

def host_prep(c, inputs):
    """Build per-core in_maps from full inputs."""
    bf = ml_dtypes.bfloat16
    f32 = np.float32
    ii = {k: np.asarray(v) for k, v in inputs.items()}
    S, S2, D, L = c.S, c.S2, c.D, c.L

    def tokmaj(vec, nt):  # [nt*128] -> [128, nt]
        return np.ascontiguousarray(vec.reshape(nt, 128).T)

    rel = ii["rel_emb"].astype(f32)  # [2*SPAN, D]

    # full-weight program tensors (rank-independent): build once
    wq_f = ii["Wq"].astype(f32)                       # [L, D, D]
    wk_f = ii["Wk"].astype(f32)
    wv_f = ii["Wv"].astype(f32)
    wqkv = np.concatenate([wq_f, wk_f, wv_f], axis=2)  # [L, D, 3D]
    wqkv = wqkv.reshape(L, c.DT, 128, 3 * D).transpose(0, 2, 1, 3)
    wqkv = np.ascontiguousarray(wqkv.astype(bf))

    bq = ii["bq"].astype(f32) * c.scale               # [L, D]
    bk = ii["bk"].astype(f32)
    bqkv = np.concatenate(
        [bq.reshape(L, c.JT, 128).transpose(0, 2, 1),
         bk.reshape(L, c.JT, 128).transpose(0, 2, 1)], axis=2)
    bqkv = np.ascontiguousarray(bqkv)
    bvrep = np.ascontiguousarray(np.broadcast_to(
        ii["bv"].astype(f32)[:, None, :], (L, 128, D)))

    wo_ = ii["Wo"].astype(f32).reshape(L, c.JT, 128, D).transpose(0, 2, 1, 3)
    wo_ = np.ascontiguousarray(wo_.astype(bf))
    bo2 = np.ascontiguousarray(
        ii["bo"].astype(f32)[:, None, :].astype(bf))

    w1_ = ii["W1"].astype(f32).reshape(L, c.DT, 128, c.F).transpose(0, 2, 1, 3)
    w1_ = np.ascontiguousarray(w1_.astype(bf))
    b1_ = np.ascontiguousarray(
        ii["b1"].astype(f32).reshape(L, c.FT, 128).transpose(0, 2, 1))
    w2_ = ii["W2"].astype(f32).reshape(L, c.FT, 128, D).transpose(0, 2, 1, 3)
    w2_ = np.ascontiguousarray(w2_.astype(bf))
    b22 = np.ascontiguousarray(
        ii["b2"].astype(f32)[:, None, :].astype(bf))

    lns = {
        k: np.ascontiguousarray(
            ii[k2].astype(f32).reshape(L, c.DT, 128).transpose(0, 2, 1))
        for k, k2 in (("ln1g", "ln1_g"), ("ln1b", "ln1_b"),
                      ("ln2g", "ln2_g"), ("ln2b", "ln2_b"))
    }

    # per-rank pos tables
    pos_tabs = {}
    for r in range(2):
        poskt = np.zeros((L, 128, c.JT, c.PW), f32)
        posqt = np.zeros((L, 128, c.JT, c.PW), f32)
        idx_k = np.clip(1023 + 512 * r - np.arange(c.PW), 0, 2 * c.SPAN - 1)
        idx_q = np.clip(np.arange(c.PW) - 511 + 512 * r, 0, 2 * c.SPAN - 1)
        for l in range(L):
            pk = rel @ wk_f[l] + ii["bk"][l].astype(f32)          # [1024, D]
            pq = (rel @ wq_f[l] + ii["bq"][l].astype(f32)) * c.scale
            for tab, idx, dst in ((pk, idx_k, poskt), (pq, idx_q, posqt)):
                ext = tab[idx]                                    # [PW, D]
                dst[l] = ext.T.reshape(c.JT, 128, c.PW).transpose(1, 0, 2)
        pos_tabs[r] = (np.ascontiguousarray(poskt.astype(bf)),
                       np.ascontiguousarray(posqt.astype(bf)))

    tok_emb_f = np.ascontiguousarray(ii["tok_emb"].astype(f32))
    seg0rep = np.ascontiguousarray(
        np.broadcast_to(ii["seg_emb"][0].astype(f32), (128, D)))
    segdrep = np.ascontiguousarray(np.broadcast_to(
        (ii["seg_emb"][1] - ii["seg_emb"][0]).astype(f32), (128, D)))
    egrep = np.ascontiguousarray(
        np.broadcast_to(ii["emb_ln_g"].astype(f32), (128, D)))
    ebrep = np.ascontiguousarray(
        np.broadcast_to(ii["emb_ln_b"].astype(f32), (128, D)))

    in_maps = []
    for core in range(c.n_cores):
        b, r = core // 2, core % 2
        tsl = slice(r * S2, (r + 1) * S2)

        ids = ii["input_ids"][b, tsl].astype(np.int64)
        w = np.zeros((16, S2 // 16), np.int16)
        for i in range(S2):
            w[i % 16, i // 16] = ids[i]
        ids16 = np.tile(w, (8, 1))

        seg = ii["segment_ids"][b].astype(f32)
        mask = ii["attention_mask"][b].astype(f32)

        m = {
            "ids16": ids16,
            "tok_emb": tok_emb_f,
            "segsel": tokmaj(seg[tsl], c.TTq),
            "seg0rep": seg0rep,
            "segdrep": segdrep,
            "maskt": tokmaj(mask[tsl], c.TTq),
            "maskbias": tokmaj(NEG * (1.0 - mask), c.TTk),
            "egrep": egrep,
            "ebrep": ebrep,
            "poskt": pos_tabs[r][0],
            "posqt": pos_tabs[r][1],
            "wqkv": wqkv,
            "bqkv": bqkv,
            "bvrep": bvrep,
            "wo": wo_,
            "bo2": bo2,
            "w1": w1_,
            "b1": b1_,
            "w2": w2_,
            "b22": b22,
            **lns,
        }
        in_maps.append(m)
    return in_maps


def assemble(c, results):
    """results[core]["out_hT"] [128, DT, S2] -> [B, S, D] fp32."""
    out = np.zeros((c.B, c.S, c.D), np.float32)
    for b in range(c.B):
        for r in range(2):
            hT = results[2 * b + r]["out_hT"]  # [128, DT, S2]
            out[b, r * c.S2:(r + 1) * c.S2] = (
                hT.transpose(2, 1, 0).reshape(c.S2, c.D))
    return out


_nc_cache = {}


def _get_nc(c):
    key = (c.B, c.S, c.D, c.H, c.F, c.L, c.V, c.SPAN, c.n_cores, c.no_cc)
    if key not in _nc_cache:
        _nc_cache[key] = build_nc(c)
    return _nc_cache[key]


def kernel(**inputs):
    from concourse import bass_utils
    c = Cfg()
    nc = _get_nc(c)
    in_maps = host_prep(c, inputs)
    res = bass_utils.run_bass_kernel_spmd(
        nc, in_maps, core_ids=list(range(c.n_cores)))
    return assemble(c, res.results)
